# revision 14
# baseline (speedup 1.0000x reference)
"""Trainium2 Bass kernel for nn_ConvSurface: barycentric surface sampling +
3->64 linear map + ReLU + max over 24 samples.

Sharding: face dimension across 8 cores (alpha/beta/gamma shard too).
Per core: F=2048 faces x M=8 meshes (fm = m*2048 + f, mesh-major).

Device pipeline per core (bf16 compute, f32 PSUM):
  1. DMA in: corn [128,3456] f32 (layout [i,d,f,n] per partition),
     cent [128,384] f32 ([f,d]), coefa/b/g [128,3072] bf16 ([f,s]),
     wblk [6,128] bf16 (block-diag W^T x2)
  2. DVE: cd = corn - cent (3 subs, one per d) -> bf16 [i,d,f,n]
  3. DVE: dirs[d,f,s] = sum_i coef_i[f,s] * cd[i,d,f]  (per-d mults+adds;
     the t-broadcast of cd rides as a 0-step AP dim). alpha+beta+gamma=1
     folds the -center into cd.
  4. SBUF->SBUF DMA: repack dirs into PE rhs layout
     [rows 32k+3eo+d, fm_local*24] in two half-tiles (big coalesced DMAs)
  5. PE: fea = dirs . W via 4x row-tiled (32x128) bf16 matmuls, N=384
  6. Drain: mix of (A) DVE reduce_max straight from PSUM and
     (B) ACT relu-pass to SBUF bf16 + DVE pairwise-max tree
  7. DMA out bf16 [128=(eo,k), 8192=(rg,g,floc)]; host un-shuffles.
"""

import json
import sys
import types

import numpy as np

sys.path.insert(0, "/opt/trn_rl_repo")

NUM_MESHES = 8
NUM_FACES = 16384
NUM_KERNEL = 64
N_CORES = 8

F = NUM_FACES // N_CORES          # 2048 faces per core
FM = NUM_MESHES * F               # 16384 face-mesh pairs per core
FL = FM // 128                    # 128 fm-items per partition
S = 24

N_MM = 384                        # 16 faces x 24 samples per matmul
FACES_PER_MM = 16
RHS_FREE = 8 * FL * S             # rhs half-tile free size 24576
MM_PER_HALF_RG = (8 * 128 * S) // N_MM  # 64
DRAIN_A_EVERY = 11                # every Nth psum-pair drained on DVE directly


# --------------------------------------------------------------------------
# Harness patches (wait-split for walrus 1-wait limit; NTFF profiling shim)
# --------------------------------------------------------------------------

def _split_waits(bir: dict) -> dict:
    """walrus codegen accepts at most 1 sync wait per instruction (2 for
    EventSemaphore); Tile sometimes emits more. Move the excess onto NoOp
    carriers inserted just before the instruction on the same engine."""
    n = [0]
    for fn in bir.get("functions", []):
        for bb in fn.get("blocks", []):
            out = []
            for inst in bb.get("instructions", []):
                si = inst.get("sync_info") or {}
                waits = si.get("on_wait") or []
                cap = 2 if inst.get("opcode") == "EventSemaphore" else 1
                if len(waits) > cap:
                    for w in waits[cap:]:
                        n[0] += 1
                        out.append({
                            "name": f"wsplit-{n[0]}",
                            "opcode": "NoOp",
                            "engine": inst.get("engine"),
                            "ins": [], "outs": [],
                            "debug": inst.get("debug"),
                            "sync_info": {"on_update": [], "on_wait": [w]},
                        })
                    si["on_wait"] = waits[:cap]
                    inst["sync_info"] = si
                out.append(inst)
            bb["instructions"] = out
    return bir


def _install_patches():
    import concourse.bass_utils as bu
    import concourse.bass2jax as b2j
    if not getattr(bu, "_wsplit_installed", False):
        orig = bu.compile_bir_kernel

        def wrapper(bir_str, *a, **kw):
            if isinstance(bir_str, (bytes, bytearray)):
                bir_str = json.dumps(_split_waits(json.loads(bir_str))).encode()
            elif isinstance(bir_str, str):
                bir_str = json.dumps(_split_waits(json.loads(bir_str)))
            return orig(bir_str, *a, **kw)

        bu.compile_bir_kernel = wrapper
        b2j.compile_bir_kernel = wrapper
        bu._wsplit_installed = True

    if "antenv.axon_hooks" not in sys.modules:
        mod = types.ModuleType("antenv.axon_hooks")
        _hook = [None]
        mod.set_axon_ntff_profile_hook = lambda h: _hook.__setitem__(0, h)
        mod.get_axon_ntff_profile_hook = lambda: _hook[0]
        sys.modules["antenv.axon_hooks"] = mod
        try:
            import antenv
            antenv.axon_hooks = mod
            from trn_agent_boot.trn_boot import _ntff_profile_via_ctypes
            mod.set_axon_ntff_profile_hook(
                _ntff_profile_via_ctypes("/opt/axon/libaxon_pjrt.so"))
        except Exception:
            pass


# --------------------------------------------------------------------------
# Device kernel
# --------------------------------------------------------------------------

def _merge_ap(ap_obj):
    """Merge adjacent free dims (outer.step == inner.step*inner.count), drop
    count-1 dims -> fit the 3-free-dim ISA mem-pattern limit."""
    import concourse.bass as bass
    pairs = [list(p) for p in ap_obj.ap]
    part, rest = pairs[0], pairs[1:]
    merged = []
    for s, c in rest:
        if c == 1:
            continue
        if merged and merged[-1][0] == s * c:
            merged[-1] = [s, merged[-1][1] * c]
        else:
            merged.append([s, c])
    if not merged:
        merged = [[1, 1]]
    return bass.AP(ap_obj.tensor, ap_obj.offset, [part] + merged)


def _build_nc():
    import concourse.bass as bass
    import concourse.tile as tile
    from concourse import mybir

    f32 = mybir.dt.float32
    bf16 = mybir.dt.bfloat16
    nc = bass.Bass()

    corn_d = nc.declare_dram_parameter("corn", [128, FL * 27], bf16, isOutput=False)
    cent_d = nc.declare_dram_parameter("cent", [128, FL * 3], bf16, isOutput=False)
    coef_d = [nc.declare_dram_parameter(f"coef{i}", [128, FL * S], bf16,
                                        isOutput=False) for i in range(3)]
    wblk_d = nc.declare_dram_parameter("wblk", [6, 128], bf16, isOutput=False)
    out_d = nc.declare_dram_parameter("out", [128, FM // 2], bf16, isOutput=True)

    AX = mybir.AluOpType

    with tile.TileContext(nc) as tc:
        with (
            tc.tile_pool(name="inputs", bufs=1) as inp_pool,
            tc.tile_pool(name="w", bufs=1) as w_pool,
            tc.tile_pool(name="dirs", bufs=1) as dirs_pool,
            tc.tile_pool(name="tmp", bufs=1) as tmp_pool,
            tc.tile_pool(name="rhs", bufs=2) as rhs_pool,
            tc.tile_pool(name="fsb", bufs=2) as fsb_pool,
            tc.tile_pool(name="tree", bufs=2) as tree_pool,
            tc.tile_pool(name="osb", bufs=2) as osb_pool,
            tc.tile_pool(name="psum", bufs=2, space="PSUM") as psum_pool,
        ):
            # ---- loads -------------------------------------------------
            corn = inp_pool.tile([128, FL * 27], bf16)    # [i, d, f, n]
            nc.sync.dma_start(corn[:], corn_d[:])
            cent = inp_pool.tile([128, FL * 3], bf16)     # [f, d]
            nc.sync.dma_start(cent[:], cent_d[:])
            coef = []
            for i in range(3):
                t = inp_pool.tile([128, FL * S], bf16, tag=f"coef{i}")  # [f, s]
                nc.sync.dma_start(t[:], coef_d[i][:])
                coef.append(t)
            wt = w_pool.tile([128, 128], bf16)
            for rg in range(4):
                nc.sync.dma_start(wt[32 * rg:32 * rg + 6, :], wblk_d[:, :])

            # ---- cd = corn - cent  (bf16, layout [i, d, f, n]) ---------
            cr5 = corn[:].rearrange("p (i d f n) -> p i d f n", i=3, d=3, f=FL, n=3)
            cd5 = cr5
            ce3 = cent[:].rearrange("p (f d) -> p f d", f=FL, d=3)
            for d in range(3):
                ce = ce3[:, :, d].unsqueeze(1).unsqueeze(3)
                ce = ce.broadcast_to((128, 3, FL, 3))
                nc.vector.tensor_tensor(
                    _merge_ap(cd5[:, :, d, :, :]), _merge_ap(cr5[:, :, d, :, :]),
                    _merge_ap(ce), op=AX.subtract)

            # ---- dirs[d, f, s] = sum_i coef_i[f,s] * cd[i,d,f] ---------
            dirs = []
            for d in range(3):
                dt_ = dirs_pool.tile([128, FL * S], bf16, tag=f"dirs{d}")
                dirs.append(dt_)
            for d in range(3):

                def cd_ap(i):
                    a = cd5[:, i, d, :, :]                 # p f n
                    a = a.unsqueeze(2).broadcast_to((128, FL, 8, 3))
                    return _merge_ap(a)                    # (f:3,t:0,n:1)

                dsl = dirs[d][:]
                t1 = tmp_pool.tile([128, FL * S], bf16, tag="t1")
                nc.vector.tensor_mul(t1[:], coef[0][:], cd_ap(0))
                t2 = tmp_pool.tile([128, FL * S], bf16, tag="t2")
                nc.vector.tensor_mul(t2[:], coef[1][:], cd_ap(1))
                nc.vector.tensor_add(t1[:], t1[:], t2[:])
                t2b = tmp_pool.tile([128, FL * S], bf16, tag="t2")
                nc.vector.tensor_mul(t2b[:], coef[2][:], cd_ap(2))
                nc.vector.tensor_add(dsl, t1[:], t2b[:])

            # ---- halves: repack + matmuls + drain ----------------------
            pair_idx = 0
            for h in range(2):
                rhs = rhs_pool.tile([128, RHS_FREE], bf16)
                # src partitions {32k+16eo+8h+j}, j<8 -> dst row 32k+3eo+d
                for eo in range(2):
                    for d in range(3):
                        for k in range(4):
                            src = dirs[d][32 * k + 16 * eo + 8 * h:
                                          32 * k + 16 * eo + 8 * h + 8, :]
                            dst = rhs[32 * k + 3 * eo + d:32 * k + 3 * eo + d + 1, :]
                            dst = dst.rearrange("p (j c) -> p j c", j=8, c=FL * S)
                            nc.sync.dma_start(dst, src)

                for rg in range(4):
                    osb = osb_pool.tile([128, 1024], bf16)
                    for b in range(8):          # 8 batches x 2 quads x 4 MMs
                        is_a = (b % 5 == 4)
                        fsb = None if is_a else fsb_pool.tile([128, 3072], bf16)
                        for q in range(2):
                            ps = psum_pool.tile([128, 2048], f32)
                            for u in range(4):
                                j = (b * 2 + q) * 4 + u
                                nc.tensor.matmul(
                                    ps[:, u * 512:u * 512 + N_MM],
                                    wt[32 * rg:32 * rg + 6, :],
                                    rhs[32 * rg:32 * rg + 6,
                                        j * N_MM:(j + 1) * N_MM],
                                    start=True, stop=True,
                                    tile_position=(32 * rg, 0))
                            if is_a:
                                pa = bass.AP(
                                    ps[:].tensor, ps[:].offset,
                                    [list(ps[:].ap[0]),
                                     [512, 4], [S, FACES_PER_MM], [1, S]])
                                osl = osb[:, (b * 2 + q) * 64:(b * 2 + q + 1) * 64]
                                nc.vector.tensor_reduce(
                                    osl.rearrange("p (u f) -> p u f", u=4),
                                    pa, axis=mybir.AxisListType.X, op=AX.max)
                            else:
                                pa = bass.AP(
                                    ps[:].tensor, ps[:].offset,
                                    [list(ps[:].ap[0]), [512, 4], [1, N_MM]])
                                nc.scalar.activation(
                                    fsb[:, q * 1536:(q + 1) * 1536].rearrange(
                                        "p (u c) -> p u c", u=4),
                                    pa, mybir.ActivationFunctionType.Relu)
                        if not is_a:
                            f3 = fsb[:].rearrange("p (g s) -> p g s", g=128, s=S)
                            tr1 = tree_pool.tile([128, 1536], bf16, tag="tr1")
                            nc.vector.tensor_tensor(
                                tr1[:], _merge_ap(f3[:, :, 0:12]),
                                _merge_ap(f3[:, :, 12:24]), op=AX.max)
                            t13 = tr1[:].rearrange("p (g s) -> p g s", g=128, s=12)
                            tr2 = tree_pool.tile([128, 768], bf16, tag="tr2")
                            nc.vector.tensor_tensor(
                                tr2[:], _merge_ap(t13[:, :, 0:6]),
                                _merge_ap(t13[:, :, 6:12]), op=AX.max)
                            t23 = tr2[:].rearrange("p (g s) -> p g s", g=128, s=6)
                            tr3 = tree_pool.tile([128, 384], bf16, tag="tr3")
                            nc.vector.tensor_tensor(
                                tr3[:], _merge_ap(t23[:, :, 0:3]),
                                _merge_ap(t23[:, :, 3:6]), op=AX.max)
                            nc.vector.tensor_reduce(
                                osb[:, b * 128:(b + 1) * 128],
                                tr3[:].rearrange("p (g s) -> p g s", g=128, s=3),
                                axis=mybir.AxisListType.X, op=AX.max)
                    nc.vector.tensor_scalar_max(osb[:], osb[:], 0.0)
                    nc.sync.dma_start(
                        out_d[:, (rg * 2 + h) * 1024:(rg * 2 + h + 1) * 1024],
                        osb[:])
    return nc


_CACHE = {}


def _get_nc():
    if "nc" not in _CACHE:
        _install_patches()
        _CACHE["nc"] = _build_nc()
    return _CACHE["nc"]


# --------------------------------------------------------------------------
# Host wrapper
# --------------------------------------------------------------------------

def _prep_core_inputs(centers, neighbor_corners, alpha, beta, gamma, W, c):
    import ml_dtypes
    bf = ml_dtypes.bfloat16
    fsl = slice(c * F, (c + 1) * F)
    cent = np.ascontiguousarray(
        centers[:, fsl].reshape(128, FL, 3),
        dtype=np.float32).reshape(128, FL * 3).astype(bf)
    # corn per-partition rows [f, n, i, d] -> [i, d, f, n]
    cr = neighbor_corners[:, fsl].reshape(128, FL, 3, 3, 3)
    corn = np.ascontiguousarray(cr.transpose(0, 3, 4, 1, 2), dtype=np.float32)
    corn = corn.reshape(128, FL * 27).astype(bf)
    cf = []
    for arr in (alpha, beta, gamma):
        a = np.tile(arr[fsl].reshape(1, F, S), (NUM_MESHES, 1, 1))
        cf.append(np.ascontiguousarray(
            a.reshape(128, FL * S), dtype=np.float32).astype(bf))
    wblk = np.zeros((6, 128), dtype=np.float32)
    wblk[0:3, 0:64] = W.T
    wblk[3:6, 64:128] = W.T
    return {"corn": corn, "cent": cent,
            "coef0": cf[0], "coef1": cf[1], "coef2": cf[2],
            "wblk": wblk.astype(bf)}


def _unshuffle_core_out(raw):
    # raw [128=(eo,k), 8192=(rg,h,g',floc)] -> [8 meshes, 2048 faces, 64]
    # col = rg*2048 + h*1024 + g'*128 + floc, where g = h*8+g'
    r = np.asarray(raw, dtype=np.float32).reshape(2, 64, 4, 16, 128)
    r = r.transpose(2, 0, 3, 4, 1)              # rg,eo,g,floc,k
    return r.reshape(NUM_MESHES, F, NUM_KERNEL)


def run(inputs, trace=False):
    from concourse.bass_utils import run_bass_kernel_spmd
    nc = _get_nc()
    centers = np.asarray(inputs["centers"], dtype=np.float32)
    corners = np.asarray(inputs["neighbor_corners"], dtype=np.float32)
    alpha = np.asarray(inputs["alpha"], dtype=np.float32)
    beta = np.asarray(inputs["beta"], dtype=np.float32)
    gamma = np.asarray(inputs["gamma"], dtype=np.float32)
    W = np.asarray(inputs["W"], dtype=np.float32)

    in_maps = [
        _prep_core_inputs(centers, corners, alpha, beta, gamma, W, c)
        for c in range(N_CORES)
    ]
    res = run_bass_kernel_spmd(
        nc, in_maps, core_ids=list(range(N_CORES)), trace=trace)
    out = np.empty((NUM_MESHES, NUM_FACES, NUM_KERNEL), dtype=np.float32)
    for c in range(N_CORES):
        out[:, c * F:(c + 1) * F, :] = _unshuffle_core_out(res.results[c]["out"])
    return out, res


def kernel(**inputs) -> np.ndarray:
    out, _ = run(inputs, trace=False)
    return out
